# revision 1
# baseline (speedup 1.0000x reference)
# RBF Gram matrix kernel for Trainium2 (8 NeuronCores, SPMD).
#
# reference:  G[i, j] = exp(-gamma * ||x_i - y_j||^2)
#                    = exp(2*gamma*((x@y^T)[i,j] - 0.5*||y_j||^2) - gamma*||x_i||^2)
#
# Sharding: row-shard x across 8 cores (1024 rows each), replicate y.
# Each core computes a [1024, 8192] slice of G:
#   PE   : xy = x_c @ y^T       (bf16 in, fp32 PSUM, K=512 as 4 k-tiles)
#   DVE  : s  = xy + (-0.5*||y||^2)  (free-dim broadcast row, one wide op)
#   ACT  : o  = Exp(2*gamma*s + (-gamma*||x||^2))  (per-partition bias)
#   DMA  : o tile (bf16) -> DRAM; host upcasts to fp32
#
# x^T / y^T are shipped pre-permuted into the exact SBUF images so every
# prefetch chunk is one contiguous DMA.
import os

import numpy as np
import ml_dtypes

N_CORES = 8
N_FULL = 8192          # rows of x (and of G)
M_FULL = 8192          # rows of y (cols of G)
D = 512                # feature dim (contraction)
MC = N_FULL // N_CORES # 1024 rows of x per core
P = 128                # SBUF partitions
NT = 512               # moving-dim tile (max; one fp32 psum bank)
KT = D // P            # 4 k-tiles
MT = MC // P           # 8 m-tiles per core
NG = 1024              # psum group width: 2 banks
NGR = M_FULL // NG     # 8 n-groups

_cache = {}


def _build_program(scale2g: float, mc: int, n_full: int, d: int):
    """Build + compile the SPMD bass program. scale2g = 2*gamma immediate."""
    import concourse.mybir as mybir
    import concourse.tile as tile
    from concourse import bacc

    mt = mc // P
    kt = d // P
    ng_w = min(NG, n_full)
    ngroups = n_full // ng_w
    nnsub = ng_w // NT

    nc = bacc.Bacc("TRN2", target_bir_lowering=False, debug=False,
                   num_devices=N_CORES)

    # exact SBUF images (see kernel() for the host-side permutes)
    xT_d = nc.dram_tensor("xTb", [P, mt * kt * P], mybir.dt.bfloat16,
                          kind="ExternalInput").ap()
    yT_d = nc.dram_tensor("yTb", [P, ngroups * kt * ng_w], mybir.dt.bfloat16,
                          kind="ExternalInput").ap()
    y2_d = nc.dram_tensor("y2n", [1, n_full], mybir.dt.float32,
                          kind="ExternalInput").ap()
    x2_d = nc.dram_tensor("x2b", [P, mt], mybir.dt.float32,
                          kind="ExternalInput").ap()
    out_d = nc.dram_tensor("out", [mc, n_full], mybir.dt.bfloat16,
                           kind="ExternalOutput").ap()

    with tile.TileContext(nc) as tc:
        with (
            tc.tile_pool(name="resident", bufs=1) as res_pool,
            tc.tile_pool(name="psum", bufs=4, space="PSUM") as psum_pool,
            tc.tile_pool(name="sq", bufs=4) as s_pool,
            tc.tile_pool(name="ot", bufs=6) as o_pool,
        ):
            xT_sb = res_pool.tile([P, mt * kt * P], mybir.dt.bfloat16, tag="xT")
            yT_sb = res_pool.tile([P, ngroups * kt * ng_w], mybir.dt.bfloat16,
                                  tag="yT")
            y2r_sb = res_pool.tile([1, n_full], mybir.dt.float32, tag="y2r")
            y2_sb = res_pool.tile([P, n_full], mybir.dt.float32, tag="y2")
            x2_sb = res_pool.tile([P, mt], mybir.dt.float32, tag="x2")
            scr_sb = res_pool.tile([P, 2 * P], mybir.dt.bfloat16, tag="scr")

            def lhsT(k, m):
                c0 = (m * kt + k) * P
                return xT_sb[:, c0:c0 + P]

            def rhs(k, ng, nn):
                c0 = ((ng * nnsub + nn) * kt + k) * NT
                return yT_sb[:, c0:c0 + NT]

            # PE warm-up: short matmuls on zeroed scratch keep the HAM
            # activity window busy while the startup DMAs land, so the
            # real matmuls run at 2.4 GHz from the first one. The psum
            # slot is recycled by the pool afterwards.
            nc.vector.memset(scr_sb, 0.0)
            wps = psum_pool.tile([P, ng_w], mybir.dt.float32,
                                 name="wps", tag="ps")
            for _ in range(24):
                nc.tensor.matmul(wps[:, 0:P], lhsT=scr_sb[:, P:2 * P],
                                 rhs=scr_sb[:, 0:P], start=True, stop=True)

            def bcast_y2(ch):
                sl = slice(ch * ng_w, (ch + 1) * ng_w)
                nc.gpsimd.partition_broadcast(y2_sb[:, sl], y2r_sb[0:1, sl])

            def dma_yt_block(b):
                c0 = b * kt * NT
                nc.sync.dma_start(out=yT_sb[:, c0:c0 + kt * NT],
                                  in_=yT_d[:, c0:c0 + kt * NT])

            def dma_yt_chunk(ch):
                for bb in range(nnsub):
                    dma_yt_block(ch * nnsub + bb)

            # startup set, in critical-path order. The first y^T block is
            # split per k so its four 128KB pieces ride parallel DMA queues.
            nc.sync.dma_start(out=xT_sb[:, 0:kt * P], in_=xT_d[:, 0:kt * P])
            for k in range(kt):
                c0 = k * NT
                nc.sync.dma_start(out=yT_sb[:, c0:c0 + NT],
                                  in_=yT_d[:, c0:c0 + NT])
            nc.sync.dma_start(out=y2r_sb, in_=y2_d)
            nc.sync.dma_start(out=x2_sb, in_=x2_d)
            for bb in range(1, nnsub):
                dma_yt_block(bb)
            bcast_y2(0)
            if mt > 1:
                nc.sync.dma_start(out=xT_sb[:, kt * P:2 * kt * P],
                                  in_=xT_d[:, kt * P:2 * kt * P])
            if mt > 2:
                nc.sync.dma_start(out=xT_sb[:, 2 * kt * P:],
                                  in_=xT_d[:, 2 * kt * P:])
            if ngroups > 1:
                dma_yt_chunk(1)
                bcast_y2(1)

            for ng in range(ngroups):
                gsl = slice(ng * ng_w, (ng + 1) * ng_w)
                if ng + 2 < ngroups:
                    dma_yt_chunk(ng + 2)   # just-in-time prefetch
                    bcast_y2(ng + 2)
                for m in range(mt):
                    msl = slice(m * P, (m + 1) * P)
                    ps = psum_pool.tile([P, ng_w], mybir.dt.float32, tag="ps")
                    for k in range(kt):
                        for nn in range(nnsub):
                            nc.tensor.matmul(
                                ps[:, nn * NT:(nn + 1) * NT],
                                lhsT=lhsT(k, m),
                                rhs=rhs(k, ng, nn),
                                start=(k == 0),
                                stop=(k == kt - 1),
                            )
                    last = (ng == ngroups - 1) and (m == mt - 1)
                    if not last:
                        s = s_pool.tile([P, ng_w], mybir.dt.float32)
                        nc.vector.tensor_add(s, ps, y2_sb[:, gsl])
                        o = o_pool.tile([P, ng_w], mybir.dt.bfloat16)
                        nc.scalar.activation(
                            o, s, mybir.ActivationFunctionType.Exp,
                            bias=x2_sb[:, m:m + 1], scale=float(scale2g),
                        )
                        nc.sync.dma_start(out=out_d[msl, gsl], in_=o)
                    else:
                        # split the final drain chain to shorten the tail
                        for nn in range(nnsub):
                            nsl = slice(ng * ng_w + nn * NT,
                                        ng * ng_w + (nn + 1) * NT)
                            psl = slice(nn * NT, (nn + 1) * NT)
                            s = s_pool.tile([P, NT], mybir.dt.float32,
                                            name=f"sl{nn}", tag=f"sl{nn}")
                            nc.vector.tensor_add(s, ps[:, psl], y2_sb[:, nsl])
                            o = o_pool.tile([P, NT], mybir.dt.bfloat16,
                                            name=f"ol{nn}", tag=f"ol{nn}")
                            nc.scalar.activation(
                                o, s, mybir.ActivationFunctionType.Exp,
                                bias=x2_sb[:, m:m + 1], scale=float(scale2g),
                            )
                            nc.sync.dma_start(out=out_d[msl, nsl], in_=o)

    nc.compile()
    return nc


def _build_program_raw(scale2g: float, mc: int, n_full: int, d: int):
    """Raw-Bass build: explicit per-engine programs + hand-rolled semaphores.
    Avoids the Tile scheduler's ~7us prologue and ~10us exit butterfly."""
    from contextlib import ExitStack, contextmanager

    import concourse.bass as bass
    import concourse.mybir as mybir
    from concourse import bacc

    class _NoBarrierBlock(bass.BassBlock):
        """BassBlock whose exit emits per-engine drains but no all-engine
        barrier; cross-engine ordering is fully covered by our semaphores."""

        def __exit__(self, exc_type, exc_val, exc_tb):
            if exc_type is not None:
                return
            for engine, last_body in self.last_body.items():
                with self.bass.body(last_body, parent=self.bass.cur_bb,
                                    allow_existing_parent=True):
                    engine.br(self.end_bb)
            self.bass.switch_bb(self.end_bb)
            gpsimd_type = self.bass.gpsimd.engine
            for eng_type, eng in self.bass.engines.items():
                if eng_type == gpsimd_type:
                    continue
                dr = mybir.InstDrain(
                    name=self.bass.get_next_instruction_name(),
                    ins=[], outs=[], bass_is_fusable=False)
                dr.engine = eng_type
                eng.add_instruction(dr)

    @contextmanager
    def _no_barrier_block(nc):
        assert nc.cur_block is None
        blk = _NoBarrierBlock(nc, f"block_{nc.next_id()}")
        nc.cur_block = blk
        try:
            with blk:
                yield blk
        finally:
            nc.cur_block = None

    mt = mc // P
    kt = d // P
    ng_w = min(NG, n_full)
    ngroups = n_full // ng_w
    nnsub = ng_w // NT
    G = ngroups * mt
    S_SLOTS = 4            # psum slots (8 banks / 2)
    SS_SLOTS = 4           # SBUF s-staging slots (same ring as psum)
    O_SLOTS = 8            # output staging slots
    NWARM = 24

    nc = bacc.Bacc("TRN2", target_bir_lowering=False, debug=False,
                   num_devices=N_CORES)

    xT_d = nc.dram_tensor("xTb", [P, mt * kt * P], mybir.dt.bfloat16,
                          kind="ExternalInput").ap()
    yT_d = nc.dram_tensor("yTb", [P, ngroups * kt * ng_w], mybir.dt.bfloat16,
                          kind="ExternalInput").ap()
    y2_d = nc.dram_tensor("y2n", [P, n_full], mybir.dt.float32,
                          kind="ExternalInput").ap()
    x2_d = nc.dram_tensor("x2b", [P, mt], mybir.dt.float32,
                          kind="ExternalInput").ap()
    out_d = nc.dram_tensor("out", [mc, n_full], mybir.dt.bfloat16,
                           kind="ExternalOutput").ap()

    with ExitStack() as ctx:
        ec = ctx.enter_context
        xT_sb = ec(nc.sbuf_tensor([P, mt * kt * P], mybir.dt.bfloat16))
        yT_sb = ec(nc.sbuf_tensor([P, ngroups * kt * ng_w], mybir.dt.bfloat16))
        y2_sb = ec(nc.sbuf_tensor([P, n_full], mybir.dt.float32))
        x2_sb = ec(nc.sbuf_tensor([P, mt], mybir.dt.float32))
        scr_sb = ec(nc.sbuf_tensor([P, 2 * P], mybir.dt.bfloat16))
        s_sb = ec(nc.sbuf_tensor([P, SS_SLOTS * ng_w], mybir.dt.float32))
        o_sb = ec(nc.sbuf_tensor([P, O_SLOTS * ng_w], mybir.dt.bfloat16))
        ps = ec(nc.psum_tensor([P, S_SLOTS * ng_w], mybir.dt.float32))

        s_scr = ec(nc.semaphore(name="s_scr"))
        s_xT = [ec(nc.semaphore(name=f"s_xT{i}")) for i in range(3)]
        s_yb = [ec(nc.semaphore(name=f"s_yb{i}"))
                for i in range(ngroups * nnsub)]
        yb_cnt = [0] * (ngroups * nnsub)
        s_y2c = [ec(nc.semaphore(name=f"s_y2c{i}")) for i in range(ngroups)]
        s_x2 = ec(nc.semaphore(name="s_x2"))
        s_mm = ec(nc.semaphore(name="s_mm"))
        s_dve = ec(nc.semaphore(name="s_dve"))
        s_act = ec(nc.semaphore(name="s_act"))
        s_osl = [ec(nc.semaphore(name=f"s_osl{i}")) for i in range(O_SLOTS)]

        def lhsT(k, m):
            c0 = (m * kt + k) * P
            return xT_sb[:, c0:c0 + P]

        def rhs(k, ng, nn):
            c0 = ((ng * nnsub + nn) * kt + k) * NT
            return yT_sb[:, c0:c0 + NT]

        with _no_barrier_block(nc) as block:

            def dma_chunk(sync, ci, split_first=False):
                for bb in range(nnsub):
                    b = ci * nnsub + bb
                    b0 = b * kt * NT
                    if split_first:
                        for k in range(kt):
                            sync.dma_start(
                                out=yT_sb[:, b0 + k * NT:b0 + (k + 1) * NT],
                                in_=yT_d[:, b0 + k * NT:b0 + (k + 1) * NT]
                            ).then_inc(s_yb[b], 16)
                            yb_cnt[b] += 16
                    else:
                        sync.dma_start(out=yT_sb[:, b0:b0 + kt * NT],
                                       in_=yT_d[:, b0:b0 + kt * NT]
                                       ).then_inc(s_yb[b], 16)
                        yb_cnt[b] += 16
                g0 = ci * ng_w
                sync.dma_start(out=y2_sb[:, g0:g0 + ng_w],
                               in_=y2_d[:, g0:g0 + ng_w]
                               ).then_inc(s_y2c[ci], 16)

            @block.sync
            def _(sync):
                sync.dma_start(out=xT_sb[:, 0:kt * P],
                               in_=xT_d[:, 0:kt * P]).then_inc(s_xT[0], 16)
                dma_chunk(sync, 0, split_first=True)
                sync.dma_start(out=x2_sb[:], in_=x2_d).then_inc(s_x2, 16)
                if mt > 1:
                    sync.dma_start(out=xT_sb[:, kt * P:2 * kt * P],
                                   in_=xT_d[:, kt * P:2 * kt * P]
                                   ).then_inc(s_xT[1], 16)
                if mt > 2:
                    sync.dma_start(out=xT_sb[:, 2 * kt * P:],
                                   in_=xT_d[:, 2 * kt * P:]).then_inc(s_xT[2], 16)
                if ngroups > 1:
                    dma_chunk(sync, 1)
                for ng in range(ngroups):
                    if ng + 2 < ngroups:
                        dma_chunk(sync, ng + 2)
                    gsl = slice(ng * ng_w, (ng + 1) * ng_w)
                    for m in range(mt):
                        g = ng * mt + m
                        sl = g % O_SLOTS
                        msl = slice(m * P, (m + 1) * P)
                        if g < G - 1:
                            sync.wait_ge(s_act, g + 1)
                            sync.dma_start(
                                out=out_d[msl, gsl],
                                in_=o_sb[:, sl * ng_w:(sl + 1) * ng_w]
                            ).then_inc(s_osl[sl], 16)
                        else:
                            for nn in range(nnsub):
                                sync.wait_ge(s_act, g + nn + 1)
                                sync.dma_start(
                                    out=out_d[msl,
                                              ng * ng_w + nn * NT:
                                              ng * ng_w + (nn + 1) * NT],
                                    in_=o_sb[:, sl * ng_w + nn * NT:
                                             sl * ng_w + (nn + 1) * NT]
                                ).then_inc(s_osl[sl], 16)
                # the end-of-block DRAIN quiesces the DGE queues, so no
                # explicit waits on the final transfer completions here

            @block.tensor
            def _(tensor):
                tensor.wait_ge(s_scr, 1)
                for _ in range(NWARM):
                    tensor.matmul(ps[:, 0:P], lhsT=scr_sb[:, P:2 * P],
                                  rhs=scr_sb[:, 0:P], start=True, stop=True)
                tensor.wait_ge(s_xT[0], 16)
                for ng in range(ngroups):
                    for m in range(mt):
                        g = ng * mt + m
                        sl = g % S_SLOTS
                        if ng == 0 and m == 1 and mt > 1:
                            tensor.wait_ge(s_xT[1], 16)
                        if ng == 0 and m == 2 and mt > 2:
                            tensor.wait_ge(s_xT[2], 16)
                        if g >= S_SLOTS:
                            tensor.wait_ge(s_dve, g - S_SLOTS + 1)
                        for nn in range(nnsub):
                            if m == 0:
                                b = ng * nnsub + nn
                                tensor.wait_ge(s_yb[b], yb_cnt[b])
                            for k in range(kt):
                                inst = tensor.matmul(
                                    ps[:, sl * ng_w + nn * NT:
                                       sl * ng_w + (nn + 1) * NT],
                                    lhsT=lhsT(k, m),
                                    rhs=rhs(k, ng, nn),
                                    start=(k == 0),
                                    stop=(k == kt - 1),
                                )
                        inst.then_inc(s_mm, 1)

            @block.vector
            def _(vector):
                vector.memset(scr_sb[:], 0.0).then_inc(s_scr, 1)
                for ng in range(ngroups):
                    gsl = slice(ng * ng_w, (ng + 1) * ng_w)
                    for m in range(mt):
                        g = ng * mt + m
                        sl = g % S_SLOTS
                        ssl = g % SS_SLOTS
                        vector.wait_ge(s_mm, g + 1)
                        if m == 0:
                            vector.wait_ge(s_y2c[ng], 16)
                        if g >= SS_SLOTS:
                            vector.wait_ge(s_act, g - SS_SLOTS + 1)
                        if g < G - 1:
                            vector.tensor_add(
                                s_sb[:, ssl * ng_w:(ssl + 1) * ng_w],
                                ps[:, sl * ng_w:(sl + 1) * ng_w],
                                y2_sb[:, gsl]).then_inc(s_dve, 1)
                        else:
                            # split the final drain chain to shorten the tail
                            for nn in range(nnsub):
                                vector.tensor_add(
                                    s_sb[:, ssl * ng_w + nn * NT:
                                         ssl * ng_w + (nn + 1) * NT],
                                    ps[:, sl * ng_w + nn * NT:
                                       sl * ng_w + (nn + 1) * NT],
                                    y2_sb[:, ng * ng_w + nn * NT:
                                          ng * ng_w + (nn + 1) * NT]
                                ).then_inc(s_dve, 1)

            @block.scalar
            def _(scalar):
                scalar.wait_ge(s_x2, 16)
                for ng in range(ngroups):
                    for m in range(mt):
                        g = ng * mt + m
                        ssl = g % SS_SLOTS
                        osl = g % O_SLOTS
                        q = (g - osl) // O_SLOTS
                        if q >= 1:
                            scalar.wait_ge(s_osl[osl], 16 * q)
                        if g < G - 1:
                            scalar.wait_ge(s_dve, g + 1)
                            scalar.activation(
                                o_sb[:, osl * ng_w:(osl + 1) * ng_w],
                                s_sb[:, ssl * ng_w:(ssl + 1) * ng_w],
                                mybir.ActivationFunctionType.Exp,
                                bias=x2_sb[:, m:m + 1],
                                scale=float(scale2g)).then_inc(s_act, 1)
                        else:
                            for nn in range(nnsub):
                                scalar.wait_ge(s_dve, g + nn + 1)
                                scalar.activation(
                                    o_sb[:, osl * ng_w + nn * NT:
                                         osl * ng_w + (nn + 1) * NT],
                                    s_sb[:, ssl * ng_w + nn * NT:
                                         ssl * ng_w + (nn + 1) * NT],
                                    mybir.ActivationFunctionType.Exp,
                                    bias=x2_sb[:, m:m + 1],
                                    scale=float(scale2g)).then_inc(s_act, 1)

        nc.compile()
    return nc


def _pack_xT(x_b: np.ndarray) -> np.ndarray:
    """[MC, D] bf16 -> SBUF image [128, MT*KT*128], block (m,k) at col
    (m*KT+k)*128 with element [p, c] = x[m*128 + c, k*128 + p]."""
    mcc, d = x_b.shape
    mt, kt = mcc // P, d // P
    a = x_b.reshape(mt, P, kt, P)          # [m, c, k, p]
    a = a.transpose(3, 0, 2, 1)            # [p, m, k, c]
    return np.ascontiguousarray(a.reshape(P, mt * kt * P))


def _pack_yT(y_b: np.ndarray, cw: int) -> np.ndarray:
    """[M, D] bf16 -> SBUF image [128, (M//cw)*KT*cw], block (b,k) at col
    (b*KT+k)*cw with element [p, c] = y[b*cw + c, k*128 + p]."""
    m, d = y_b.shape
    nb, kt = m // cw, d // P
    a = y_b.reshape(nb, cw, kt, P)         # [b, c, k, p]
    a = a.transpose(3, 0, 2, 1)            # [p, b, k, c]
    return np.ascontiguousarray(a.reshape(P, nb * kt * cw))


def kernel(x: np.ndarray, y: np.ndarray, gamma: np.ndarray) -> np.ndarray:
    from concourse.bass_utils import run_bass_kernel_spmd

    x = np.asarray(x, dtype=np.float32)
    y = np.asarray(y, dtype=np.float32)
    g = float(np.asarray(gamma))

    n, d = x.shape
    m = y.shape[0]
    assert (n, d, m) == (N_FULL, D, M_FULL), (n, d, m)

    raw = bool(int(os.environ.get("RBF_RAW", "1")))
    key = (g, n, d, m, raw)
    if key not in _cache:
        _cache.clear()
        build = _build_program_raw if raw else _build_program
        _cache[key] = build(2.0 * g, MC, M_FULL, D)
    nc = _cache[key]

    # host-side prep (O(N*D), ~0.01% of kernel FLOPs)
    bf16 = ml_dtypes.bfloat16
    x_b = x.astype(bf16)
    yTb = _pack_yT(y.astype(bf16), NT)
    y2 = np.einsum("md,md->m", y, y, dtype=np.float64)
    y2row = (-0.5 * y2).astype(np.float32)
    if raw:
        y2n = np.ascontiguousarray(np.broadcast_to(y2row, (P, m)))
    else:
        y2n = np.ascontiguousarray(y2row[None, :])
    x2 = np.einsum("nd,nd->n", x, x, dtype=np.float64)

    in_maps = []
    for c in range(N_CORES):
        sl = slice(c * MC, (c + 1) * MC)
        x2_c = np.ascontiguousarray(
            (-g * x2[sl]).astype(np.float32).reshape(MT, P).T)      # [128, MT]
        in_maps.append({"xTb": _pack_xT(x_b[sl]), "yTb": yTb,
                        "y2n": y2n, "x2b": x2_c})

    trace = bool(int(os.environ.get("RBF_TRACE", "0")))
    res = run_bass_kernel_spmd(nc, in_maps, core_ids=list(range(N_CORES)),
                               trace=trace)
    global LAST_RESULTS
    LAST_RESULTS = res
    return np.concatenate(
        [r["out"].astype(np.float32) for r in res.results], axis=0)


LAST_RESULTS = None



# revision 12
# speedup vs baseline: 1.5785x; 1.5785x over previous
# RBF Gram matrix kernel for Trainium2 (8 NeuronCores, SPMD).
#
# reference:  G[i, j] = exp(-gamma * ||x_i - y_j||^2)
#
# Factorization used on device:
#   G[i, j] = exp(2*gamma*xy[i,j] - gamma*||x_i||^2) * exp(-gamma*||y_j||^2)
#             \------------- ACT (bias per row i) --/   \-- DVE row mult --/
#
# which maps perfectly onto the engines:
#   PE  : xy = x_c @ y^T in fp8(e4m3) with perf_mode=DoubleRow
#         (K_virt=256 per MM -> ~1.8x bf16 MM throughput)
#   ACT : o = Exp(scale*psum + bias_i), PSUM->SBUF bf16, 2048-wide chunks
#         (bias = -gamma*||x_i||^2 is per-partition, so no DVE add needed)
#   DVE : o2 = o * c_j  (c_j = exp(-gamma*||y_j||^2) row, bf16 2x mode)
#   DMA : o2 (fp8) -> DRAM; host upcasts to fp32
#
# Sharding: row-shard x across 8 cores (1024 rows each), replicate y.
#
# Note: the exp/exp split assumes the intermediate exp(2g*xy - g*x2) does
# not overflow, which holds for the standardized inputs this kernel serves
# (|2g*xy| << g*x2). Inputs are quantized to fp8 at scale 16; the 1/256
# factor is folded into the ACT scale immediate.
import os

import numpy as np
import ml_dtypes

N_CORES = 8
N_FULL = 8192          # rows of x (and of G)
M_FULL = 8192          # rows of y (cols of G)
D = 512                # feature dim (contraction)
MC = N_FULL // N_CORES # 1024 rows of x per core
P = 128                # SBUF partitions
KT = D // P            # 4 k-subtiles of 128
MT = MC // P           # 8 m-tiles per core
CW = 2048              # chunk width (ACT/DVE/psum-slot granularity)
JC = M_FULL // CW      # 4 j-chunks
NN = CW // 512         # 4 matmul slices of 512 per chunk
G = JC * MT            # 32 chunks per core
XS = 16.0              # fp8 input scale (folded out via ACT scale)

_cache = {}


def _build_program(scale_imm: float, out_fp8: bool, dve_mode: str):
    """Raw-Bass build: explicit per-engine programs + hand-rolled semaphores."""
    from contextlib import ExitStack, contextmanager

    import concourse.bass as bass
    import concourse.mybir as mybir
    from concourse import bacc

    class _NoBarrierBlock(bass.BassBlock):
        """BassBlock whose exit emits per-engine drains but no all-engine
        barrier; cross-engine ordering is fully covered by our semaphores."""

        def __exit__(self, exc_type, exc_val, exc_tb):
            if exc_type is not None:
                return
            for engine, last_body in self.last_body.items():
                with self.bass.body(last_body, parent=self.bass.cur_bb,
                                    allow_existing_parent=True):
                    engine.br(self.end_bb)
            self.bass.switch_bb(self.end_bb)
            gpsimd_type = self.bass.gpsimd.engine
            for eng_type, eng in self.bass.engines.items():
                if eng_type == gpsimd_type:
                    continue
                dr = mybir.InstDrain(
                    name=self.bass.get_next_instruction_name(),
                    ins=[], outs=[], bass_is_fusable=False)
                dr.engine = eng_type
                eng.add_instruction(dr)

    @contextmanager
    def _no_barrier_block(nc):
        assert nc.cur_block is None
        blk = _NoBarrierBlock(nc, f"block_{nc.next_id()}")
        nc.cur_block = blk
        try:
            with blk:
                yield blk
        finally:
            nc.cur_block = None

    NWARM = 36
    O_SLOTS = 4            # ACT output staging slots (bf16)
    O2_SLOTS = 6           # DVE output staging slots (fp8/bf16)
    fp8 = mybir.dt.float8e4
    odt = mybir.dt.float8e4 if out_fp8 else mybir.dt.bfloat16
    two_step = dve_mode == "mul16copy8" and out_fp8

    nc = bacc.Bacc("TRN2", target_bir_lowering=False, debug=False,
                   num_devices=N_CORES)

    x_d = nc.dram_tensor("x8", [P, MT * KT, P], fp8,
                         kind="ExternalInput").ap()
    y_d = nc.dram_tensor("y8", [P, JC * KT, CW], fp8,
                         kind="ExternalInput").ap()
    c_d = nc.dram_tensor("cb", [P, M_FULL], mybir.dt.bfloat16,
                         kind="ExternalInput").ap()
    x2_d = nc.dram_tensor("x2b", [P, MT], mybir.dt.float32,
                          kind="ExternalInput").ap()
    out_d = nc.dram_tensor("out", [MC, M_FULL], odt,
                           kind="ExternalOutput").ap()

    with ExitStack() as ctx:
        ec = ctx.enter_context
        x_sb = ec(nc.sbuf_tensor([P, MT * KT, P], fp8))
        y_sb = ec(nc.sbuf_tensor([P, JC * KT, CW], fp8))
        c_sb = ec(nc.sbuf_tensor([P, M_FULL], mybir.dt.bfloat16))
        x2_sb = ec(nc.sbuf_tensor([P, MT], mybir.dt.float32))
        scr_sb = ec(nc.sbuf_tensor([P, 2 * P], mybir.dt.bfloat16))
        o_sb = ec(nc.sbuf_tensor([P, O_SLOTS, CW], mybir.dt.bfloat16))
        o2_sb = ec(nc.sbuf_tensor([P, O2_SLOTS, CW], odt))
        o3_sb = (ec(nc.sbuf_tensor("o3_sb", [P, CW], mybir.dt.bfloat16))
                 if two_step else None)
        ps = ec(nc.psum_tensor([P, 2, CW], mybir.dt.float32))

        s_scr = ec(nc.semaphore(name="s_scr"))
        s_x = ec(nc.semaphore(name="s_x"))
        s_x2 = ec(nc.semaphore(name="s_x2"))
        s_y = [ec(nc.semaphore(name=f"s_y{i}")) for i in range(JC)]
        s_y0k = [ec(nc.semaphore(name=f"s_y0k{i}")) for i in range(KT)]
        s_c = [ec(nc.semaphore(name=f"s_c{i}")) for i in range(JC)]
        s_mm = ec(nc.semaphore(name="s_mm"))
        s_act = ec(nc.semaphore(name="s_act"))
        s_dve = ec(nc.semaphore(name="s_dve"))
        s_osl = [ec(nc.semaphore(name=f"s_osl{i}")) for i in range(O2_SLOTS)]

        def lhsT(m, ko):
            return x_sb[:, m * KT + 2 * ko:m * KT + 2 * ko + 2, :]

        def rhs(jc, ko, nn):
            return y_sb[:, jc * KT + 2 * ko:jc * KT + 2 * ko + 2,
                        nn * 512:(nn + 1) * 512]

        with _no_barrier_block(nc) as block:

            @block.sync
            def _(sync):
                # startup set, in critical-path order; y chunk 0 is split
                # per k-subtile so its pieces ride parallel DMA queues and
                # the PE can start after the first two land.
                sync.dma_start(out=x_sb[:], in_=x_d).then_inc(s_x, 16)
                for kt in range(KT):
                    sync.dma_start(out=y_sb[:, kt:kt + 1, :],
                                   in_=y_d[:, kt:kt + 1, :]
                                   ).then_inc(s_y0k[kt], 16)
                sync.dma_start(out=x2_sb[:], in_=x2_d).then_inc(s_x2, 16)
                sync.dma_start(out=c_sb[:, 0:CW],
                               in_=c_d[:, 0:CW]).then_inc(s_c[0], 16)
                sync.dma_start(out=y_sb[:, KT:2 * KT, :],
                               in_=y_d[:, KT:2 * KT, :]).then_inc(s_y[1], 16)
                sync.dma_start(out=c_sb[:, CW:2 * CW],
                               in_=c_d[:, CW:2 * CW]).then_inc(s_c[1], 16)
                for g in range(G):
                    jc, m = g // MT, g % MT
                    # just-in-time prefetch of later y/c chunks
                    if g == 2:
                        sync.dma_start(out=y_sb[:, 2 * KT:3 * KT, :],
                                       in_=y_d[:, 2 * KT:3 * KT, :]
                                       ).then_inc(s_y[2], 16)
                    if g == 4:
                        sync.dma_start(out=c_sb[:, 2 * CW:3 * CW],
                                       in_=c_d[:, 2 * CW:3 * CW]
                                       ).then_inc(s_c[2], 16)
                    if g == 10:
                        sync.dma_start(out=y_sb[:, 3 * KT:4 * KT, :],
                                       in_=y_d[:, 3 * KT:4 * KT, :]
                                       ).then_inc(s_y[3], 16)
                    if g == 12:
                        sync.dma_start(out=c_sb[:, 3 * CW:4 * CW],
                                       in_=c_d[:, 3 * CW:4 * CW]
                                       ).then_inc(s_c[3], 16)
                    sl = g % O2_SLOTS
                    msl = slice(m * P, (m + 1) * P)
                    if g < G - 1:
                        sync.wait_ge(s_dve, g + 1)
                        sync.dma_start(
                            out=out_d[msl, jc * CW:(jc + 1) * CW],
                            in_=o2_sb[:, sl, :]).then_inc(s_osl[sl], 16)
                    else:
                        # split the final drain chain to shorten the tail
                        for nn in range(NN):
                            sync.wait_ge(s_dve, g + nn + 1)
                            sync.dma_start(
                                out=out_d[msl, jc * CW + nn * 512:
                                          jc * CW + (nn + 1) * 512],
                                in_=o2_sb[:, sl, nn * 512:(nn + 1) * 512]
                            ).then_inc(s_osl[sl], 16)
                # the end-of-block DRAIN quiesces the DGE queues

            @block.tensor
            def _(tensor):
                # PE warm-up: keep the HAM activity window busy while the
                # startup DMAs land, so real matmuls run at 2.4 GHz.
                tensor.wait_ge(s_scr, 1)
                for _ in range(NWARM):
                    tensor.matmul(ps[:, 0, 0:P], lhsT=scr_sb[:, P:2 * P],
                                  rhs=scr_sb[:, 0:P], start=True, stop=True)
                tensor.wait_ge(s_x, 16)
                for g in range(G):
                    jc, m = g // MT, g % MT
                    sl = g % 2
                    if g >= 2:
                        tensor.wait_ge(s_act, g - 1)   # psum slot free
                    if m == 0 and jc > 0:
                        tensor.wait_ge(s_y[jc], 16)
                    for ko in range(2):
                        if g == 0:
                            # per-kt gating: the 4 startup y DMAs ride
                            # parallel queues and can complete out of order
                            tensor.wait_ge(s_y0k[2 * ko], 16)
                            tensor.wait_ge(s_y0k[2 * ko + 1], 16)
                        for nn in range(NN):
                            inst = tensor.matmul(
                                ps[:, sl, nn * 512:(nn + 1) * 512],
                                lhsT=lhsT(m, ko),
                                rhs=rhs(jc, ko, nn),
                                start=(ko == 0),
                                stop=(ko == 1),
                                perf_mode=mybir.MatmulPerfMode.DoubleRow,
                            )
                    inst.then_inc(s_mm, 1)

            @block.scalar
            def _(scalar):
                # dummy activation so the one-time exp table load (~2.7us)
                # overlaps the startup DMAs instead of the first real chunk.
                # o_sb slot 0 is private to this engine until chunk 0.
                scalar.activation(o_sb[:, 0, 0:2], o_sb[:, 0, 0:2],
                                  mybir.ActivationFunctionType.Exp)
                scalar.wait_ge(s_x2, 16)
                for g in range(G):
                    jc, m = g // MT, g % MT
                    osl = g % O_SLOTS
                    if g >= O_SLOTS:
                        scalar.wait_ge(s_dve, g - O_SLOTS + 1)  # o slot free
                    if g < G - 1:
                        scalar.wait_ge(s_mm, g + 1)
                        scalar.activation(
                            o_sb[:, osl, :], ps[:, g % 2, :],
                            mybir.ActivationFunctionType.Exp,
                            bias=x2_sb[:, m:m + 1],
                            scale=float(scale_imm)).then_inc(s_act, 1)
                    else:
                        scalar.wait_ge(s_mm, g + 1)
                        for nn in range(NN):
                            scalar.activation(
                                o_sb[:, osl, nn * 512:(nn + 1) * 512],
                                ps[:, g % 2, nn * 512:(nn + 1) * 512],
                                mybir.ActivationFunctionType.Exp,
                                bias=x2_sb[:, m:m + 1],
                                scale=float(scale_imm)).then_inc(s_act, 1)

            @block.vector
            def _(vector):
                vector.memset(scr_sb[:], 0.0).then_inc(s_scr, 1)
                for g in range(G):
                    jc, m = g // MT, g % MT
                    osl = g % O_SLOTS
                    sl = g % O2_SLOTS
                    q = g // O2_SLOTS
                    if q >= 1:
                        vector.wait_ge(s_osl[sl], 16 * q)   # o2 slot free
                    if m == 0:
                        vector.wait_ge(s_c[jc], 16)
                    csl = c_sb[:, jc * CW:(jc + 1) * CW]
                    if g < G - 1:
                        vector.wait_ge(s_act, g + 1)
                        if two_step:
                            # keep the multiply in the bf16 2x mode; the
                            # fp8 downcast rides the (faster) copy uop
                            vector.tensor_mul(o3_sb[:], o_sb[:, osl, :], csl)
                            vector.tensor_copy(o2_sb[:, sl, :],
                                               o3_sb[:]).then_inc(s_dve, 1)
                        else:
                            vector.tensor_mul(o2_sb[:, sl, :],
                                              o_sb[:, osl, :],
                                              csl).then_inc(s_dve, 1)
                    else:
                        for nn in range(NN):
                            nsl = slice(nn * 512, (nn + 1) * 512)
                            vector.wait_ge(s_act, g + nn + 1)
                            if two_step:
                                vector.tensor_mul(o3_sb[:, nsl],
                                                  o_sb[:, osl, nsl],
                                                  csl[:, nsl])
                                vector.tensor_copy(
                                    o2_sb[:, sl, nsl],
                                    o3_sb[:, nsl]).then_inc(s_dve, 1)
                            else:
                                vector.tensor_mul(
                                    o2_sb[:, sl, nsl], o_sb[:, osl, nsl],
                                    csl[:, nsl]).then_inc(s_dve, 1)

        nc.compile()
    return nc


def _pack_xT(x_8: np.ndarray) -> np.ndarray:
    """[MC, D] fp8 -> SBUF image [128, MT*KT, 128]; k-subtile kt of m-tile m
    at dim1 index m*KT+kt with element [p, ., c] = x[m*128 + c, kt*128 + p]."""
    mcc, d = x_8.shape
    mt, kt = mcc // P, d // P
    a = x_8.reshape(mt, P, kt, P)          # [m, c, kt, p]
    a = a.transpose(3, 0, 2, 1)            # [p, m, kt, c]
    return np.ascontiguousarray(a.reshape(P, mt * kt, P))


def _pack_yT(y_8: np.ndarray) -> np.ndarray:
    """[M, D] fp8 -> SBUF image [128, JC*KT, CW]; k-subtile kt of j-chunk jc
    at dim1 index jc*KT+kt with element [p, ., c] = y[jc*CW + c, kt*128 + p]."""
    m, d = y_8.shape
    jc, kt = m // CW, d // P
    a = y_8.reshape(jc, CW, kt, P)         # [jc, c, kt, p]
    a = a.transpose(3, 0, 2, 1)            # [p, jc, kt, c]
    return np.ascontiguousarray(a.reshape(P, jc * kt, CW))


def kernel(x: np.ndarray, y: np.ndarray, gamma: np.ndarray) -> np.ndarray:
    from concourse.bass_utils import run_bass_kernel_spmd

    x = np.asarray(x, dtype=np.float32)
    y = np.asarray(y, dtype=np.float32)
    g = float(np.asarray(gamma))

    n, d = x.shape
    m = y.shape[0]
    assert (n, d, m) == (N_FULL, D, M_FULL), (n, d, m)

    out_fp8 = os.environ.get("RBF_OUT", "fp8") == "fp8"
    dve_mode = os.environ.get("RBF_DVE", "mul16copy8")
    scale_imm = 2.0 * g / (XS * XS)
    key = (g, out_fp8, dve_mode)
    if key not in _cache:
        _cache.clear()
        _cache[key] = _build_program(scale_imm, out_fp8, dve_mode)
    nc = _cache[key]

    # host-side prep (O(N*D), ~0.01% of kernel FLOPs)
    f8 = ml_dtypes.float8_e4m3fn
    x8 = np.clip(x * XS, -240.0, 240.0).astype(f8)
    y8 = np.clip(y * XS, -240.0, 240.0).astype(f8)
    y_img = _pack_yT(y8)
    x2 = np.einsum("nd,nd->n", x, x, dtype=np.float64)
    y2 = np.einsum("md,md->m", y, y, dtype=np.float64)
    c_row = np.exp(-g * y2).astype(ml_dtypes.bfloat16)
    c_rep = np.ascontiguousarray(np.broadcast_to(c_row[None, :], (P, m)))

    in_maps = []
    for c in range(N_CORES):
        sl = slice(c * MC, (c + 1) * MC)
        x2_c = np.ascontiguousarray(
            (-g * x2[sl]).astype(np.float32).reshape(MT, P).T)   # [128, MT]
        in_maps.append({"x8": _pack_xT(x8[sl]), "y8": y_img,
                        "cb": c_rep, "x2b": x2_c})

    trace = bool(int(os.environ.get("RBF_TRACE", "0")))
    res = run_bass_kernel_spmd(nc, in_maps, core_ids=list(range(N_CORES)),
                               trace=trace)
    global LAST_RESULTS
    LAST_RESULTS = res
    return np.concatenate(
        [r["out"].astype(np.float32) for r in res.results], axis=0)


LAST_RESULTS = None
